# revision 16
# baseline (speedup 1.0000x reference)
"""Trainium2 Bass kernel for nn_DGM_77318001263213 (dense_transformer).

Reference computation (per batch b of 16):
  dir_map = conv3x3_SAME(x, dc_w) + dc_b            [12, 64, 64]
  q = conv2x2_s2(dir_map, q_w) + q_b  -> [48, 1024]
  k = conv2x2_s2(dir_map, k_w) + k_b  -> [48, 1024]
  v = conv2x2_s2(x, v_w) + v_b        -> [1024, 1024]
  attn = softmax(q^T k, axis=-1)                    [1024, 1024]
  out[c, m] = sum_n v[c, n] * attn[m, n]            [1024, 1024]

Device mapping (data-parallel, 2 batches per core on 8 cores):
  * q,k computed as ONE composite 4x4 stride-2 convolution of x (dc conv
    folded with the proj convs on the host), stacked M=96, in fp32r --
    the scores S = q^T k need ~1e-3 absolute logit precision, so the
    q/k path stays in fp32r.
  * v conv + attention-weighted sum run in bf16 (global rel err ~3.5e-3,
    verified by host simulation): bf16 stationaries on [128,128] tiles
    get FWL (fast weight load), and the bf16 tiles halve SBUF/DMA.
  * space-to-depth x for the v conv is precomputed on the host in bf16
    and DMA'd on the (otherwise idle) vector HWDGE queue.
  * scores computed transposed T[n, m] = S[m, n]; E = exp(T) (no max
    subtraction; |S| <= ~25 fits bf16/f32 range), unnormalized
    U^T[m, c] = sum_n E[n, m] V^T[n, c], row sums D[m] via ones-matmul
    on esum partials, out^T[m, c] = U^T[m, c] / D[m] on DVE.
  * DMA order is consumption order: wqk groups 0-1, first x plane, the
    rest of wqk, remaining planes, then wv; batch 1's planes prefetch
    during batch 0's compute so the PE never goes idle between batches
    (PE idle > ~3.4us triggers HAM re-throttle to 1.2GHz).
  * single shared 8-bank PSUM pool rotated by all matmul groups.
  * host transposes out^T -> out at gather time.
"""
import os
import sys
import types
import numpy as np
from contextlib import ExitStack

for _p in ("/opt/trn_rl_repo", "/root/.axon_site/_ro/trn_rl_repo"):
    if os.path.isdir(_p) and _p not in sys.path:
        sys.path.insert(0, _p)

import ml_dtypes

import concourse.bacc as bacc
import concourse.bass as bass
import concourse.tile as tile
import concourse.mybir as mybir
from concourse import bass_utils

F32 = mybir.dt.float32
F32R = mybir.dt.float32r
FP16 = mybir.dt.float16
BF16 = mybir.dt.bfloat16
ts = bass.ts

NCORES = 8
BPC = 2          # batches per core
C_IN = 256
NPOS = 1024      # 32*32 output positions
OC = 1024        # v output channels


def _install_ntff_hook_shim():
    """Register the axon NTFF profile hook if the image's antenv lacks it.

    Only needed when BASS_TRACE=1; harmless otherwise."""
    if "antenv.axon_hooks" in sys.modules:
        return
    try:
        from trn_agent_boot.trn_boot import _ntff_profile_via_ctypes
        hook = _ntff_profile_via_ctypes("/opt/axon/libaxon_pjrt.so")
    except Exception:
        hook = None
    m = types.ModuleType("antenv.axon_hooks")
    m.get_axon_ntff_profile_hook = lambda: hook
    m.set_axon_ntff_profile_hook = lambda h: None
    sys.modules["antenv.axon_hooks"] = m


def build_program():
    """Build the per-core Bacc program (same program on all 8 cores)."""
    nc = bacc.Bacc(trn_type="TRN2", target_bir_lowering=False, debug=False)

    # padded x as 4 stride-2 parity planes: xq[b, c, a*2+p, r, s] =
    # x_pad[b, c, 2r+a, 2s+p] -- stride-1 innermost free dim for the PE
    xq = nc.dram_tensor("xq", [BPC, C_IN, 4, 33, 33], BF16, kind="ExternalInput")
    # wqk chunk order = (h, pl, u, v) == qk conv consumption order
    wqk = nc.dram_tensor("wqk", [128, 32, 96], BF16, kind="ExternalInput")
    wv = nc.dram_tensor("wv", [128, 8, OC], BF16, kind="ExternalInput")
    bqk = nc.dram_tensor("bqk", [96, 1], F32, kind="ExternalInput")
    bvr = nc.dram_tensor("bvr", [128, OC], BF16, kind="ExternalInput")
    o = nc.dram_tensor("o", [BPC, NPOS, OC], BF16, kind="ExternalOutput")

    EXP = mybir.ActivationFunctionType.Exp

    with tile.TileContext(nc) as tc, ExitStack() as ctx:
        const = ctx.enter_context(tc.tile_pool(name="const", bufs=1))
        xpool = ctx.enter_context(tc.tile_pool(name="xpool", bufs=8))
        xspool = ctx.enter_context(tc.tile_pool(name="xspool", bufs=16))
        qkp = ctx.enter_context(tc.tile_pool(name="qkp", bufs=1))
        epool = ctx.enter_context(tc.tile_pool(name="epool", bufs=1))
        vtpool = ctx.enter_context(tc.tile_pool(name="vtpool", bufs=1))
        outp = ctx.enter_context(tc.tile_pool(name="outp", bufs=4))
        misc = ctx.enter_context(tc.tile_pool(name="misc", bufs=2))
        pp = ctx.enter_context(tc.tile_pool(name="pp", bufs=8, space="PSUM"))

        def ptile():
            return pp.tile([128, 512], F32, tag="pp", name="pp")

        # ---- DMA schedule: need-time ordered across sync/scalar/gpsimd ----
        # per-queue bandwidth is ~100-155GB/s when several queues are active;
        # everything the qk conv needs is interleaved across the two HWDGE
        # queues in consumption order, v-path weights ride the SWDGE queue.
        xh = [[[None] * 4 for _ in range(2)] for _ in range(BPC)]

        def load_plane(b, h, pl, eng):
            t = xpool.tile([128, 33, 33], BF16, tag="xh", name="xh")
            eng.dma_start(t[:], xq.ap()[b, ts(h, 128), pl])
            xh[b][h][pl] = t

        wqk_c = []

        def load_wqk(i, lo, hi, eng):
            t = const.tile([128, hi - lo, 96], BF16, tag=f"wqk_c{i}", name=f"wqk_c{i}")
            eng.dma_start(t[:], wqk.ap()[:, lo:hi, :])
            wqk_c.append((lo, hi, t))

        def wqk_view(tap):
            for lo, hi, t in wqk_c:
                if lo <= tap < hi:
                    return t[:, tap - lo, :]

        # ---- PE warm-up: ~3.5us of dummy matmuls on a zeroed tile, no
        # DMA dependency, so HAM reaches the 2.4GHz p-state before the
        # first real matmul's data lands ----
        warm_sb = const.tile([128, 512], BF16, tag="warm_sb")
        nc.vector.memset(warm_sb[:], 0.0)
        for wi in range(10):
            pw_t = ptile()
            nc.tensor.matmul(
                pw_t[:], warm_sb[:, 0:128], warm_sb[:],
                start=True, stop=True,
            )

        # just-in-time DMA order. Engine DMAs fan out to concurrent hw
        # queues (~240GB/s aggregate), so the early in-flight set must be
        # ONLY what the first taps need: c0+p000 on sync, c1+p001 on
        # scalar, the remaining wqk chunks on the SWDGE queue, planes
        # staggered 2-per-engine matching the conv's ~1.9us/plane pace.
        load_wqk(0, 0, 4, nc.sync)
        load_plane(0, 0, 0, nc.sync)
        load_wqk(1, 4, 8, nc.scalar)
        load_plane(0, 0, 1, nc.scalar)
        for i in range(2, 8):
            load_wqk(i, 4 * i, 4 * i + 4, nc.gpsimd)
        load_plane(0, 0, 2, nc.sync)
        load_plane(0, 0, 3, nc.scalar)
        load_plane(0, 1, 0, nc.sync)
        load_plane(0, 1, 1, nc.scalar)
        load_plane(0, 1, 2, nc.sync)
        load_plane(0, 1, 3, nc.scalar)
        bqk_sb = const.tile([96, 1], F32, tag="bqk_sb")
        nc.scalar.dma_start(bqk_sb[:], bqk.ap())
        wv_sb = []
        wt0 = const.tile([128, 8, 512], BF16, tag="wv_sb0", name="wv_sb0")
        nc.gpsimd.dma_start(wt0[:], wv.ap()[:, :, 0:512])
        wv_sb.append(wt0)
        bvr_sb = const.tile([128, OC], BF16, tag="bvr_sb")
        nc.gpsimd.dma_start(bvr_sb[:], bvr.ap())
        wt1 = const.tile([128, 8, 512], BF16, tag="wv_sb1", name="wv_sb1")
        nc.scalar.dma_start(wt1[:], wv.ap()[:, :, 512:1024])
        wv_sb.append(wt1)
        # batch 1 planes: backpressured by the 8-slot plane pool, so their
        # issue waits for batch 0's releases
        for h in range(2):
            for pl in range(4):
                load_plane(1, h, pl, nc.sync if pl % 2 == 0 else nc.scalar)

        # N=2 ones for the D-sum matmuls (bf16 to match e_sb stationaries
        # is not needed -- D works on the f32r esum accumulator)
        ones_f32 = const.tile([128, 2], F32, tag="ones_f32")
        nc.vector.memset(ones_f32[:], 1.0)
        ones2 = const.tile([128, 2], F32R, tag="ones2")
        nc.scalar.copy(ones2[:], ones_f32[:])

        for b in range(BPC):
            # ---- composite q|k conv, jm-inner so planes are consumed in
            #      DMA arrival order (one plane per 8 matmuls ~2us) ----
            pq_t = [ptile() for _ in range(2)]
            for tap in range(32):
                h, rem = divmod(tap, 16)
                pl, uv = divmod(rem, 4)
                u, v = divmod(uv, 2)
                wview = wqk_view(tap)
                for jm in range(2):
                    rhs = xh[b][h][pl][:, u + 16 * jm : u + 16 * jm + 16, v : v + 32]
                    nc.tensor.matmul(
                        pq_t[jm][:96, :], wview, rhs,
                        start=(tap == 0), stop=(tap == 31),
                    )
            QK = qkp.tile([96, NPOS], FP16, tag="QK")
            for jm in range(2):
                nc.vector.tensor_scalar_add(
                    QK[:, ts(jm, 512)], pq_t[jm][:96, :], bqk_sb[:, :1]
                )
            # K-padded score operands: lhsT rows 48:128 are zero so the
            # scores matmuls keep the same 128-row PE array config as their
            # neighbors (row-group changes cost ~100ns per matmul); the
            # moving operand rows 48:128 are garbage multiplied by zero
            Ktp = qkp.tile([128, NPOS], FP16, tag="Ktp")
            nc.vector.memset(Ktp[:], 0.0)
            for jm in range(2):
                nc.vector.tensor_copy(Ktp[0:48, ts(jm, 512)], QK[0:48, ts(jm, 512)])
            Qs = qkp.tile([128, NPOS], FP16, tag="Qs")
            nc.vector.memset(Qs[:], 0.0)
            nc.gpsimd.dma_start(Qs[0:48, :], QK[48:96, :])

            # ---- space-to-depth x (bf16), derived on device ----
            xs_c = [None] * 8
            for ck in (6, 7, 4, 5, 2, 3, 0, 1):
                t, h = divmod(ck, 2)
                dy, dx = divmod(t, 2)
                a, u2 = (dy + 1) % 2, (dy + 1) // 2
                p2, v2 = (dx + 1) % 2, (dx + 1) // 2
                xst = xspool.tile([128, NPOS], BF16, tag="xs", name="xs")
                srcv = xh[b][h][a * 2 + p2][:, u2 : u2 + 32, v2 : v2 + 32]
                dstv = xst[:].rearrange("p (a b) -> p a b", a=32)
                if ck % 2 == 0:
                    nc.vector.tensor_copy(dstv, srcv)
                else:
                    nc.scalar.copy(dstv, srcv)
                xs_c[ck] = xst

            # ---- v conv (V^T, bf16) in four 4-bank waves (one l-half and
            #      four jn chunks each), chunk-major so the PE consumes xs
            #      chunks as they land; 4 PSUM banks stay free so the
            #      interleaved scores/exp pipeline never starves ----
            e_sb = epool.tile([128, 8, NPOS], BF16, tag="e_sb")
            vt_sb = vtpool.tile([128, 8, NPOS], BF16, tag="vt_sb")
            esum = epool.tile([128, NPOS], F32R, tag="esum")
            n_sc = 0

            def scores_step():
                nonlocal n_sc
                if n_sc >= 16:
                    return
                sn, sm = divmod(n_sc, 2)
                pt_t = ptile()
                nc.tensor.matmul(
                    pt_t[:], Ktp[:, ts(sn, 128)], Qs[:, ts(sm, 512)],
                    start=True, stop=True,
                )
                nc.scalar.activation(e_sb[:, sn, ts(sm, 512)], pt_t[:], EXP)
                if n_sc % 2 == 1:
                    if sn == 1:
                        nc.vector.tensor_add(esum[:], e_sb[:, 0, :], e_sb[:, 1, :])
                    elif sn > 1:
                        nc.vector.tensor_add(esum[:], esum[:], e_sb[:, sn, :])
                n_sc += 1

            blk = 0
            for wave in range(4):
                l, half = divmod(wave, 2)
                jns = (0, 1, 2, 3) if half == 0 else (4, 5, 6, 7)
                pv_w = {}
                for jn in jns:
                    pv_w[jn] = ptile()
                for ci, ck in enumerate((6, 7, 4, 5, 2, 3, 0, 1)):
                    for jn in jns:
                        nc.tensor.matmul(
                            pv_w[jn][:],
                            xs_c[ck][:, ts(jn, 128)],
                            wv_sb[l][:, ck, :],
                            start=(ci == 0), stop=(ci == 7),
                        )
                    # paired scores matmuls every other block (back-to-back
                    # K=48 matmuls amortize the PE row-reconfig drain)
                    if blk >= 3 and blk % 2 == 1:
                        scores_step()
                        scores_step()
                    blk += 1
                    if ci == 7:
                        for jn in jns:
                            nc.vector.tensor_add(
                                vt_sb[:, jn, ts(l, 512)], pv_w[jn][:],
                                bvr_sb[:, ts(l, 512)],
                            )

            # ---- U^T[m, c] = sum_n E[n, m] V^T[n, c]; D[m]; out^T = U^T/D ----
            for mm in range(8):
                pd_t = ptile()
                nc.tensor.matmul(
                    pd_t[:, 0:2], esum[:, ts(mm, 128)], ones2[:],
                    start=True, stop=True,
                )
                rc = misc.tile([128, 1], F32, tag="rc")
                nc.vector.reciprocal(rc[:], pd_t[:, 0:1])
                ot = outp.tile([128, OC], BF16, tag="ot")
                for l in range(2):
                    pu_t = ptile()
                    for jn in range(8):
                        nc.tensor.matmul(
                            pu_t[:],
                            e_sb[:, jn, ts(mm, 128)],
                            vt_sb[:, jn, ts(l, 512)],
                            start=(jn == 0), stop=(jn == 7),
                        )
                    nc.vector.tensor_scalar_mul(
                        ot[:, ts(l, 512)], pu_t[:], rc[:, 0:1]
                    )
                # alternate output DMAs across the two free queues; the last
                # chunks go out as halves so the final transfer tail is short
                if b == BPC - 1 and mm >= 6:
                    for qi, eng in enumerate((nc.gpsimd, nc.scalar, nc.sync, nc.gpsimd)):
                        eng.dma_start(
                            o.ap()[b, ts(mm, 128), 256 * qi : 256 * qi + 256],
                            ot[:, 256 * qi : 256 * qi + 256],
                        )
                elif b == BPC - 1 and mm >= 4:
                    nc.gpsimd.dma_start(o.ap()[b, ts(mm, 128), 0:512], ot[:, 0:512])
                    nc.scalar.dma_start(o.ap()[b, ts(mm, 128), 512:1024], ot[:, 512:1024])
                else:
                    eng = nc.gpsimd if mm % 2 == 0 else nc.scalar
                    eng.dma_start(o.ap()[b, ts(mm, 128), :], ot[:])

    nc.compile()
    return nc


def host_weights(dc_w, dc_b, q_w, k_w, q_b, k_b, v_w, v_b):
    """Fold dc conv into q/k projections -> composite 4x4 stride-2 weights."""
    dc_w = np.asarray(dc_w, np.float32)
    dc_b = np.asarray(dc_b, np.float32)
    q_w = np.asarray(q_w, np.float32)
    k_w = np.asarray(k_w, np.float32)
    q_b = np.asarray(q_b, np.float32)
    k_b = np.asarray(k_b, np.float32)
    v_w = np.asarray(v_w, np.float32)
    v_b = np.asarray(v_b, np.float32)

    C = dc_w.shape[1]
    Wq = np.zeros((48, C, 4, 4), np.float64)
    Wk = np.zeros((48, C, 4, 4), np.float64)
    for p in range(2):
        for qq in range(2):
            qw_pq = q_w[:, :, p, qq].astype(np.float64)
            kw_pq = k_w[:, :, p, qq].astype(np.float64)
            for dy in range(3):
                for dx in range(3):
                    dcw_dd = dc_w[:, :, dy, dx].astype(np.float64)
                    Wq[:, :, p + dy, qq + dx] += qw_pq @ dcw_dd
                    Wk[:, :, p + dy, qq + dx] += kw_pq @ dcw_dd
    bq_eff = q_b + q_w.sum(axis=(2, 3)) @ dc_b
    bk_eff = k_b + k_w.sum(axis=(2, 3)) @ dc_b
    # lhsT row index = (A*4+B)*C + c', columns: k 0:48 | q 48:96
    wqk_ab = (
        np.concatenate(
            [
                Wk.transpose(2, 3, 1, 0).reshape(16 * C, 48),
                Wq.transpose(2, 3, 1, 0).reshape(16 * C, 48),
            ],
            axis=1,
        )
        .astype(np.float32)
        .reshape(32, 128, 96)  # chunk_old = (A*4+B)*2 + h
    )
    # permute chunks into device consumption order (h, pl, u, v)
    perm = []
    for h in range(2):
        for pl in range(4):
            a, p = divmod(pl, 2)
            for u in range(2):
                for v in range(2):
                    A, Bo = 2 * u + a, 2 * v + p
                    perm.append((A * 4 + Bo) * 2 + h)
    wqk = wqk_ab[perm].transpose(1, 0, 2).astype(ml_dtypes.bfloat16)  # [part 128, chunk2 32, 96]
    bqk = np.concatenate([bk_eff, bq_eff]).reshape(96, 1).astype(np.float32)
    # v rhs: row = (dy*2+dx)*C + c', col = oc; bf16
    wv = np.ascontiguousarray(
        v_w.transpose(2, 3, 1, 0).reshape(8, 128, 4 * C).transpose(1, 0, 2)
    ).astype(ml_dtypes.bfloat16)  # [part 128, chunk 8, oc]
    bvr = np.ascontiguousarray(np.broadcast_to(v_b, (128, 4 * C))).astype(
        ml_dtypes.bfloat16
    )
    return wqk, bqk, wv, bvr


_PROGRAM = None
LAST_RESULTS = None


def _get_program():
    global _PROGRAM
    if _PROGRAM is None:
        _PROGRAM = build_program()
    return _PROGRAM


def kernel(x, dc_w, dc_b, q_w, q_b, k_w, k_b, v_w, v_b):
    _install_ntff_hook_shim()
    x = np.asarray(x, np.float32)
    B = x.shape[0]
    xp = np.pad(x, ((0, 0), (0, 0), (1, 1), (1, 1)))
    # parity planes: xq[b, c, a*2+p, r, s] = x_pad[b, c, 2r+a, 2s+p]
    xq = (
        xp.reshape(B, C_IN, 33, 2, 33, 2)
        .transpose(0, 1, 3, 5, 2, 4)
        .reshape(B, C_IN, 4, 33, 33)
    ).astype(ml_dtypes.bfloat16)
    wqk, bqk, wv, bvr = host_weights(dc_w, dc_b, q_w, k_w, q_b, k_b, v_w, v_b)

    nc = _get_program()
    in_maps = []
    for c in range(NCORES):
        in_maps.append(
            {
                "xq": np.ascontiguousarray(xq[BPC * c : BPC * (c + 1)]),
                "wqk": wqk,
                "wv": wv,
                "bqk": bqk,
                "bvr": bvr,
            }
        )
    res = bass_utils.run_bass_kernel_spmd(nc, in_maps, core_ids=list(range(NCORES)))
    global LAST_RESULTS
    LAST_RESULTS = res

    out = np.empty((B, 1024, 1024), np.float32)
    for c in range(NCORES):
        out[BPC * c : BPC * (c + 1)] = (
            res.results[c]["o"].astype(np.float32).transpose(0, 2, 1)
        )
    return out


# revision 17
# speedup vs baseline: 215.4164x; 215.4164x over previous
"""Trainium2 Bass kernel for nn_DGM_77318001263213 (dense_transformer).

Reference computation (per batch b of 16):
  dir_map = conv3x3_SAME(x, dc_w) + dc_b            [12, 64, 64]
  q = conv2x2_s2(dir_map, q_w) + q_b  -> [48, 1024]
  k = conv2x2_s2(dir_map, k_w) + k_b  -> [48, 1024]
  v = conv2x2_s2(x, v_w) + v_b        -> [1024, 1024]
  attn = softmax(q^T k, axis=-1)                    [1024, 1024]
  out[c, m] = sum_n v[c, n] * attn[m, n]            [1024, 1024]

Device mapping (data-parallel, 2 batches per core on 8 cores):
  * q,k computed as ONE composite 4x4 stride-2 convolution of x (dc conv
    folded with the proj convs on the host), stacked M=96, in fp32r --
    the scores S = q^T k need ~1e-3 absolute logit precision, so the
    q/k path stays in fp32r.
  * v conv + attention-weighted sum run in bf16 (global rel err ~3.5e-3,
    verified by host simulation): bf16 stationaries on [128,128] tiles
    get FWL (fast weight load), and the bf16 tiles halve SBUF/DMA.
  * space-to-depth x for the v conv is precomputed on the host in bf16
    and DMA'd on the (otherwise idle) vector HWDGE queue.
  * scores computed transposed T[n, m] = S[m, n]; E = exp(T) (no max
    subtraction; |S| <= ~25 fits bf16/f32 range), unnormalized
    U^T[m, c] = sum_n E[n, m] V^T[n, c], row sums D[m] via ones-matmul
    on esum partials, out^T[m, c] = U^T[m, c] / D[m] on DVE.
  * DMA order is consumption order: wqk groups 0-1, first x plane, the
    rest of wqk, remaining planes, then wv; batch 1's planes prefetch
    during batch 0's compute so the PE never goes idle between batches
    (PE idle > ~3.4us triggers HAM re-throttle to 1.2GHz).
  * single shared 8-bank PSUM pool rotated by all matmul groups.
  * host transposes out^T -> out at gather time.
"""
import os
import sys
import types
import numpy as np
from contextlib import ExitStack

for _p in ("/opt/trn_rl_repo", "/root/.axon_site/_ro/trn_rl_repo"):
    if os.path.isdir(_p) and _p not in sys.path:
        sys.path.insert(0, _p)

import ml_dtypes

import concourse.bacc as bacc
import concourse.bass as bass
import concourse.tile as tile
import concourse.mybir as mybir
from concourse import bass_utils

F32 = mybir.dt.float32
F32R = mybir.dt.float32r
FP16 = mybir.dt.float16
BF16 = mybir.dt.bfloat16
ts = bass.ts

NCORES = 8
BPC = 2          # batches per core
C_IN = 256
NPOS = 1024      # 32*32 output positions
OC = 1024        # v output channels


def _install_ntff_hook_shim():
    """Register the axon NTFF profile hook if the image's antenv lacks it.

    Only needed when BASS_TRACE=1; harmless otherwise."""
    if "antenv.axon_hooks" in sys.modules:
        return
    try:
        from trn_agent_boot.trn_boot import _ntff_profile_via_ctypes
        hook = _ntff_profile_via_ctypes("/opt/axon/libaxon_pjrt.so")
    except Exception:
        hook = None
    m = types.ModuleType("antenv.axon_hooks")
    m.get_axon_ntff_profile_hook = lambda: hook
    m.set_axon_ntff_profile_hook = lambda h: None
    sys.modules["antenv.axon_hooks"] = m


def build_program():
    """Build the per-core Bacc program (same program on all 8 cores)."""
    nc = bacc.Bacc(trn_type="TRN2", target_bir_lowering=False, debug=False)

    # padded x as 4 stride-2 parity planes: xq[b, c, a*2+p, r, s] =
    # x_pad[b, c, 2r+a, 2s+p] -- stride-1 innermost free dim for the PE
    xq = nc.dram_tensor("xq", [BPC, C_IN, 4, 33, 33], BF16, kind="ExternalInput")
    # wqk chunk order = (h, pl, u, v) == qk conv consumption order
    wqk = nc.dram_tensor("wqk", [128, 32, 96], BF16, kind="ExternalInput")
    wv = nc.dram_tensor("wv", [128, 8, OC], BF16, kind="ExternalInput")
    bqk = nc.dram_tensor("bqk", [96, 1], F32, kind="ExternalInput")
    bvr = nc.dram_tensor("bvr", [128, OC], BF16, kind="ExternalInput")
    o = nc.dram_tensor("o", [BPC, NPOS, OC], BF16, kind="ExternalOutput")

    EXP = mybir.ActivationFunctionType.Exp

    with tile.TileContext(nc) as tc, ExitStack() as ctx:
        const = ctx.enter_context(tc.tile_pool(name="const", bufs=1))
        xpool = ctx.enter_context(tc.tile_pool(name="xpool", bufs=8))
        xspool = ctx.enter_context(tc.tile_pool(name="xspool", bufs=16))
        qkp = ctx.enter_context(tc.tile_pool(name="qkp", bufs=1))
        epool = ctx.enter_context(tc.tile_pool(name="epool", bufs=1))
        vtpool = ctx.enter_context(tc.tile_pool(name="vtpool", bufs=1))
        outp = ctx.enter_context(tc.tile_pool(name="outp", bufs=4))
        misc = ctx.enter_context(tc.tile_pool(name="misc", bufs=2))
        pp = ctx.enter_context(tc.tile_pool(name="pp", bufs=8, space="PSUM"))

        def ptile():
            return pp.tile([128, 512], F32, tag="pp", name="pp")

        # ---- DMA schedule: need-time ordered across sync/scalar/gpsimd ----
        # per-queue bandwidth is ~100-155GB/s when several queues are active;
        # everything the qk conv needs is interleaved across the two HWDGE
        # queues in consumption order, v-path weights ride the SWDGE queue.
        xh = [[[None] * 4 for _ in range(2)] for _ in range(BPC)]

        def load_plane(b, h, pl, eng):
            t = xpool.tile([128, 33, 33], BF16, tag="xh", name="xh")
            eng.dma_start(t[:], xq.ap()[b, ts(h, 128), pl])
            xh[b][h][pl] = t

        wqk_c = []

        def load_wqk(i, lo, hi, eng):
            t = const.tile([128, hi - lo, 96], BF16, tag=f"wqk_c{i}", name=f"wqk_c{i}")
            eng.dma_start(t[:], wqk.ap()[:, lo:hi, :])
            wqk_c.append((lo, hi, t))

        def wqk_view(tap):
            for lo, hi, t in wqk_c:
                if lo <= tap < hi:
                    return t[:, tap - lo, :]

        # ---- PE warm-up: ~3.5us of dummy matmuls on a zeroed tile, no
        # DMA dependency, so HAM reaches the 2.4GHz p-state before the
        # first real matmul's data lands ----
        warm_sb = const.tile([128, 512], BF16, tag="warm_sb")
        nc.vector.memset(warm_sb[:], 0.0)
        for wi in range(10):
            pw_t = ptile()
            nc.tensor.matmul(
                pw_t[:], warm_sb[:, 0:128], warm_sb[:],
                start=True, stop=True,
            )

        # just-in-time DMA order. Engine DMAs fan out to concurrent hw
        # queues (~240GB/s aggregate), so the early in-flight set must be
        # ONLY what the first taps need: c0+p000 on sync, c1+p001 on
        # scalar, the remaining wqk chunks on the SWDGE queue, planes
        # staggered 2-per-engine matching the conv's ~1.9us/plane pace.
        load_wqk(0, 0, 4, nc.sync)
        load_plane(0, 0, 0, nc.sync)
        load_wqk(1, 4, 8, nc.scalar)
        load_plane(0, 0, 1, nc.scalar)
        for i in range(2, 8):
            load_wqk(i, 4 * i, 4 * i + 4, nc.gpsimd)
        load_plane(0, 0, 2, nc.sync)
        load_plane(0, 0, 3, nc.scalar)
        load_plane(0, 1, 0, nc.sync)
        load_plane(0, 1, 1, nc.scalar)
        load_plane(0, 1, 2, nc.sync)
        load_plane(0, 1, 3, nc.scalar)
        bqk_sb = const.tile([96, 1], F32, tag="bqk_sb")
        nc.scalar.dma_start(bqk_sb[:], bqk.ap())
        wv_sb = []
        wt0 = const.tile([128, 8, 512], BF16, tag="wv_sb0", name="wv_sb0")
        nc.gpsimd.dma_start(wt0[:], wv.ap()[:, :, 0:512])
        wv_sb.append(wt0)
        bvr_sb = const.tile([128, OC], BF16, tag="bvr_sb")
        nc.gpsimd.dma_start(bvr_sb[:], bvr.ap())
        wt1 = const.tile([128, 8, 512], BF16, tag="wv_sb1", name="wv_sb1")
        nc.scalar.dma_start(wt1[:], wv.ap()[:, :, 512:1024])
        wv_sb.append(wt1)
        # batch 1 planes: backpressured by the 8-slot plane pool (their
        # issue waits for batch 0's releases) -- they must stay on the sync
        # engine, which executes nothing the release depends on
        for h in range(2):
            for pl in range(4):
                load_plane(1, h, pl, nc.sync)

        # N=2 ones for the D-sum matmuls (bf16 to match e_sb stationaries
        # is not needed -- D works on the f32r esum accumulator)
        ones_f32 = const.tile([128, 2], F32, tag="ones_f32")
        nc.vector.memset(ones_f32[:], 1.0)
        ones2 = const.tile([128, 2], F32R, tag="ones2")
        nc.scalar.copy(ones2[:], ones_f32[:])

        for b in range(BPC):
            # ---- composite q|k conv, jm-inner so planes are consumed in
            #      DMA arrival order (one plane per 8 matmuls ~2us) ----
            pq_t = [ptile() for _ in range(2)]
            for tap in range(32):
                h, rem = divmod(tap, 16)
                pl, uv = divmod(rem, 4)
                u, v = divmod(uv, 2)
                wview = wqk_view(tap)
                for jm in range(2):
                    rhs = xh[b][h][pl][:, u + 16 * jm : u + 16 * jm + 16, v : v + 32]
                    nc.tensor.matmul(
                        pq_t[jm][:96, :], wview, rhs,
                        start=(tap == 0), stop=(tap == 31),
                    )
            QK = qkp.tile([96, NPOS], FP16, tag="QK")
            for jm in range(2):
                nc.vector.tensor_scalar_add(
                    QK[:, ts(jm, 512)], pq_t[jm][:96, :], bqk_sb[:, :1]
                )
            # K-padded score operands: lhsT rows 48:128 are zero so the
            # scores matmuls keep the same 128-row PE array config as their
            # neighbors (row-group changes cost ~100ns per matmul); the
            # moving operand rows 48:128 are garbage multiplied by zero
            Ktp = qkp.tile([128, NPOS], FP16, tag="Ktp")
            nc.vector.memset(Ktp[:], 0.0)
            for jm in range(2):
                nc.vector.tensor_copy(Ktp[0:48, ts(jm, 512)], QK[0:48, ts(jm, 512)])
            Qs = qkp.tile([128, NPOS], FP16, tag="Qs")
            nc.vector.memset(Qs[:], 0.0)
            nc.gpsimd.dma_start(Qs[0:48, :], QK[48:96, :])

            # ---- space-to-depth x (bf16), derived on device ----
            xs_c = [None] * 8
            for ck in (6, 7, 4, 5, 2, 3, 0, 1):
                t, h = divmod(ck, 2)
                dy, dx = divmod(t, 2)
                a, u2 = (dy + 1) % 2, (dy + 1) // 2
                p2, v2 = (dx + 1) % 2, (dx + 1) // 2
                xst = xspool.tile([128, NPOS], BF16, tag="xs", name="xs")
                srcv = xh[b][h][a * 2 + p2][:, u2 : u2 + 32, v2 : v2 + 32]
                dstv = xst[:].rearrange("p (a b) -> p a b", a=32)
                if ck % 2 == 0:
                    nc.vector.tensor_copy(dstv, srcv)
                else:
                    nc.scalar.copy(dstv, srcv)
                xs_c[ck] = xst

            # ---- v conv (V^T, bf16) in four 4-bank waves (one l-half and
            #      four jn chunks each), chunk-major so the PE consumes xs
            #      chunks as they land; 4 PSUM banks stay free so the
            #      interleaved scores/exp pipeline never starves ----
            e_sb = epool.tile([128, 8, NPOS], BF16, tag="e_sb")
            vt_sb = vtpool.tile([128, 8, NPOS], BF16, tag="vt_sb")
            esum = epool.tile([128, NPOS], F32R, tag="esum")
            n_sc = 0

            def scores_step():
                nonlocal n_sc
                if n_sc >= 16:
                    return
                sn, sm = divmod(n_sc, 2)
                pt_t = ptile()
                nc.tensor.matmul(
                    pt_t[:], Ktp[:, ts(sn, 128)], Qs[:, ts(sm, 512)],
                    start=True, stop=True,
                )
                nc.scalar.activation(e_sb[:, sn, ts(sm, 512)], pt_t[:], EXP)
                if n_sc % 2 == 1:
                    if sn == 1:
                        nc.vector.tensor_add(esum[:], e_sb[:, 0, :], e_sb[:, 1, :])
                    elif sn > 1:
                        nc.vector.tensor_add(esum[:], esum[:], e_sb[:, sn, :])
                n_sc += 1

            blk = 0
            for wave in range(4):
                l, half = divmod(wave, 2)
                jns = (0, 1, 2, 3) if half == 0 else (4, 5, 6, 7)
                pv_w = {}
                for jn in jns:
                    pv_w[jn] = ptile()
                for ci, ck in enumerate((6, 7, 4, 5, 2, 3, 0, 1)):
                    for jn in jns:
                        nc.tensor.matmul(
                            pv_w[jn][:],
                            xs_c[ck][:, ts(jn, 128)],
                            wv_sb[l][:, ck, :],
                            start=(ci == 0), stop=(ci == 7),
                        )
                    # paired scores matmuls every other block (back-to-back
                    # K=48 matmuls amortize the PE row-reconfig drain)
                    if blk >= 3 and blk % 2 == 1:
                        scores_step()
                        scores_step()
                    blk += 1
                    if ci == 7:
                        for jn in jns:
                            nc.vector.tensor_add(
                                vt_sb[:, jn, ts(l, 512)], pv_w[jn][:],
                                bvr_sb[:, ts(l, 512)],
                            )

            # ---- U^T[m, c] = sum_n E[n, m] V^T[n, c]; D[m]; out^T = U^T/D ----
            for mm in range(8):
                pd_t = ptile()
                nc.tensor.matmul(
                    pd_t[:, 0:2], esum[:, ts(mm, 128)], ones2[:],
                    start=True, stop=True,
                )
                rc = misc.tile([128, 1], F32, tag="rc")
                nc.vector.reciprocal(rc[:], pd_t[:, 0:1])
                ot = outp.tile([128, OC], BF16, tag="ot")
                for l in range(2):
                    pu_t = ptile()
                    for jn in range(8):
                        nc.tensor.matmul(
                            pu_t[:],
                            e_sb[:, jn, ts(mm, 128)],
                            vt_sb[:, jn, ts(l, 512)],
                            start=(jn == 0), stop=(jn == 7),
                        )
                    nc.vector.tensor_scalar_mul(
                        ot[:, ts(l, 512)], pu_t[:], rc[:, 0:1]
                    )
                # alternate output DMAs across the two free queues; the last
                # chunks go out as halves so the final transfer tail is short
                if b == BPC - 1 and mm >= 6:
                    for qi, eng in enumerate((nc.gpsimd, nc.scalar, nc.sync, nc.gpsimd)):
                        eng.dma_start(
                            o.ap()[b, ts(mm, 128), 256 * qi : 256 * qi + 256],
                            ot[:, 256 * qi : 256 * qi + 256],
                        )
                elif b == BPC - 1 and mm >= 4:
                    nc.gpsimd.dma_start(o.ap()[b, ts(mm, 128), 0:512], ot[:, 0:512])
                    nc.scalar.dma_start(o.ap()[b, ts(mm, 128), 512:1024], ot[:, 512:1024])
                else:
                    eng = nc.gpsimd if mm % 2 == 0 else nc.scalar
                    eng.dma_start(o.ap()[b, ts(mm, 128), :], ot[:])

    nc.compile()
    return nc


def host_weights(dc_w, dc_b, q_w, k_w, q_b, k_b, v_w, v_b):
    """Fold dc conv into q/k projections -> composite 4x4 stride-2 weights."""
    dc_w = np.asarray(dc_w, np.float32)
    dc_b = np.asarray(dc_b, np.float32)
    q_w = np.asarray(q_w, np.float32)
    k_w = np.asarray(k_w, np.float32)
    q_b = np.asarray(q_b, np.float32)
    k_b = np.asarray(k_b, np.float32)
    v_w = np.asarray(v_w, np.float32)
    v_b = np.asarray(v_b, np.float32)

    C = dc_w.shape[1]
    Wq = np.zeros((48, C, 4, 4), np.float64)
    Wk = np.zeros((48, C, 4, 4), np.float64)
    for p in range(2):
        for qq in range(2):
            qw_pq = q_w[:, :, p, qq].astype(np.float64)
            kw_pq = k_w[:, :, p, qq].astype(np.float64)
            for dy in range(3):
                for dx in range(3):
                    dcw_dd = dc_w[:, :, dy, dx].astype(np.float64)
                    Wq[:, :, p + dy, qq + dx] += qw_pq @ dcw_dd
                    Wk[:, :, p + dy, qq + dx] += kw_pq @ dcw_dd
    bq_eff = q_b + q_w.sum(axis=(2, 3)) @ dc_b
    bk_eff = k_b + k_w.sum(axis=(2, 3)) @ dc_b
    # lhsT row index = (A*4+B)*C + c', columns: k 0:48 | q 48:96
    wqk_ab = (
        np.concatenate(
            [
                Wk.transpose(2, 3, 1, 0).reshape(16 * C, 48),
                Wq.transpose(2, 3, 1, 0).reshape(16 * C, 48),
            ],
            axis=1,
        )
        .astype(np.float32)
        .reshape(32, 128, 96)  # chunk_old = (A*4+B)*2 + h
    )
    # permute chunks into device consumption order (h, pl, u, v)
    perm = []
    for h in range(2):
        for pl in range(4):
            a, p = divmod(pl, 2)
            for u in range(2):
                for v in range(2):
                    A, Bo = 2 * u + a, 2 * v + p
                    perm.append((A * 4 + Bo) * 2 + h)
    wqk = wqk_ab[perm].transpose(1, 0, 2).astype(ml_dtypes.bfloat16)  # [part 128, chunk2 32, 96]
    bqk = np.concatenate([bk_eff, bq_eff]).reshape(96, 1).astype(np.float32)
    # v rhs: row = (dy*2+dx)*C + c', col = oc; bf16
    wv = np.ascontiguousarray(
        v_w.transpose(2, 3, 1, 0).reshape(8, 128, 4 * C).transpose(1, 0, 2)
    ).astype(ml_dtypes.bfloat16)  # [part 128, chunk 8, oc]
    bvr = np.ascontiguousarray(np.broadcast_to(v_b, (128, 4 * C))).astype(
        ml_dtypes.bfloat16
    )
    return wqk, bqk, wv, bvr


_PROGRAM = None
LAST_RESULTS = None


def _get_program():
    global _PROGRAM
    if _PROGRAM is None:
        _PROGRAM = build_program()
    return _PROGRAM


def kernel(x, dc_w, dc_b, q_w, q_b, k_w, k_b, v_w, v_b):
    _install_ntff_hook_shim()
    x = np.asarray(x, np.float32)
    B = x.shape[0]
    xp = np.pad(x, ((0, 0), (0, 0), (1, 1), (1, 1)))
    # parity planes: xq[b, c, a*2+p, r, s] = x_pad[b, c, 2r+a, 2s+p]
    xq = (
        xp.reshape(B, C_IN, 33, 2, 33, 2)
        .transpose(0, 1, 3, 5, 2, 4)
        .reshape(B, C_IN, 4, 33, 33)
    ).astype(ml_dtypes.bfloat16)
    wqk, bqk, wv, bvr = host_weights(dc_w, dc_b, q_w, k_w, q_b, k_b, v_w, v_b)

    nc = _get_program()
    in_maps = []
    for c in range(NCORES):
        in_maps.append(
            {
                "xq": np.ascontiguousarray(xq[BPC * c : BPC * (c + 1)]),
                "wqk": wqk,
                "wv": wv,
                "bqk": bqk,
                "bvr": bvr,
            }
        )
    res = bass_utils.run_bass_kernel_spmd(nc, in_maps, core_ids=list(range(NCORES)))
    global LAST_RESULTS
    LAST_RESULTS = res

    out = np.empty((B, 1024, 1024), np.float32)
    for c in range(NCORES):
        out[BPC * c : BPC * (c + 1)] = (
            res.results[c]["o"].astype(np.float32).transpose(0, 2, 1)
        )
    return out
